# revision 30
# baseline (speedup 1.0000x reference)
"""Trainium2 Bass kernel for nn_Attention_Encode (B=4, N=2048, DIM=1024, H=16, DH=64).

Sharding: 16 heads -> 8 cores x 2 heads (tensor parallel). Each core computes
  ztu_g = W_g @ ZT^T          (its 128 output channels = 2 heads)
  attention per (batch, head) with Q=K=V=ztu
  partial_out = ssa_g @ W_g   (row-sharded output projection)
Host sums the 8 partials (the all-reduce step of a row-sharded projection).

On-device layout is fully transposed ("scoresT" = [keys, queries]) so that
softmax needs no transposes: the AV matmul's stationary operand [V | ones]
produces both the numerator and the softmax denominator.

v3 changes vs baseline:
 - QK matmuls are K=64 row-packed via tile_position: head A occupies PE
   rows 0:63, head B rows 64:127, the two matmuls run concurrently in the
   array (2x QK throughput; ztuT holds both heads stacked, unpadded).
 - Scores ring has 3 slots (6 PSUM banks); proj1/proj2/bc/warm share the
   same ring so the whole kernel fits 8 banks while QK can run 2 groups
   ahead of exp -> ACT stays saturated, PE never gates on exp latency.
 - A tunable fraction of exp tiles runs on the Vector engine as a
   Schraudolph fast-exp (one fused mult+add tensor_scalar emitting the
   bf16 bit pattern via an int16 store), offloading the saturated ACT.
 - ztuN (v-natural) is built by DMA xbar transposes (no PE/DVE involved);
   AV stationary operand is [v(64) | ones] with M=65.
 - proj2 writes both 512-column halves into one 2-bank PSUM tile: one
   bf16 cast, one contiguous 2KB-per-row DMA store per 128 rows.
"""
import os
import sys

for _p in ('/opt/trn_rl_repo',):
    if _p not in sys.path:
        sys.path.insert(0, _p)

from contextlib import ExitStack

import numpy as np
import ml_dtypes

import concourse.bacc as bacc
import concourse.mybir as mybir
import concourse.tile as tile
from concourse.bass_utils import run_bass_kernel_spmd
from concourse.masks import make_identity

B, N, C = 4, 2048, 1024          # batch, seq, model dim
KP, DH, HPER = 128, 64, 2        # per-core channels, head dim, heads per core
NQB = 512                        # query block
NKT = 128                        # key tile
NTB = N // NKT                   # 16 key tiles per batch
NTILES = B * NTB                 # 64 n-tiles total
SCALE = DH ** -0.5               # 0.125
BF = mybir.dt.bfloat16
F32 = mybir.dt.float32
F32R = mybir.dt.float32r
I16 = mybir.dt.int16

# Schraudolph fast-exp (bf16 bit pattern built by one fp32 mult+add):
#   bits_i16 = round(score * A_SCH + B_SCH);  bitcast -> bf16 ~ exp(score*SCALE)
LOG2E = 1.4426950408889634
SIGMA_SCH = 0.0579               # minimax offset, |rel err| <= ~4%
A_SCH = SCALE * LOG2E * 128.0
B_SCH = (127.0 - SIGMA_SCH) * 128.0
DVE_EXP_EVERY = int(os.environ.get("DVE_EXP_EVERY", "3"))

_CACHE = {}


def _build_kernel():
    nc = bacc.Bacc("TRN2", target_bir_lowering=False, debug=False)
    ztt = nc.dram_tensor("ztt", [B, C, N], BF, kind="ExternalInput").ap()
    wgt = nc.dram_tensor("wgt", [C, KP], BF, kind="ExternalInput").ap()   # W_g^T
    wg = nc.dram_tensor("wg", [KP, C], BF, kind="ExternalInput").ap()     # W_g
    out = nc.dram_tensor("out", [B * N, C], BF, kind="ExternalOutput").ap()

    with tile.TileContext(nc) as tc, ExitStack() as ctx:
        _body(ctx, tc, ztt, wgt, wg, out)
    nc.compile()
    return nc


def _body(ctx, tc, ztt, wgt, wg, out):
    nc = tc.nc
    singles = ctx.enter_context(tc.tile_pool(name="singles", bufs=1))
    zin_pool = ctx.enter_context(tc.tile_pool(name="zin", bufs=16))
    sc_pool = ctx.enter_context(tc.tile_pool(name="sc", bufs=3, space="PSUM"))
    av_pool = ctx.enter_context(tc.tile_pool(name="av", bufs=2, space="PSUM"))
    ex_pool = ctx.enter_context(tc.tile_pool(name="ex", bufs=12))
    sn_pool = ctx.enter_context(tc.tile_pool(name="sn", bufs=4))
    rc_pool = ctx.enter_context(tc.tile_pool(name="rc", bufs=4))

    # ---- persistent SBUF ----
    wgt_sb = singles.tile([128, 8, KP], BF)            # [c-in-tile, ci, k]
    nc.sync.dma_start(out=wgt_sb, in_=wgt.rearrange("(ci p) k -> p ci k", p=128))
    wg_sb = singles.tile([KP, C], BF)
    nc.sync.dma_start(out=wg_sb, in_=wg)
    ident = singles.tile([128, 128], BF)
    make_identity(nc, ident)
    sel = singles.tile([128, 128], BF)                 # den -> per-head row broadcast
    nc.vector.memset(sel, 0.0)
    nc.vector.memset(sel[0:1, 0:64], 1.0)
    nc.vector.memset(sel[32:33, 64:128], 1.0)
    dn = singles.tile([128, NQB], BF)                  # dens: head A row 0, head B row 32
    nc.vector.memset(dn, 0.0)
    # ztu^T with both heads stacked: rows 0:64 = head A, rows 64:128 = head B.
    # QK matmuls use K=64 slices; tile_position row-packing (auto-derived from
    # base partitions) runs the two heads' matmuls concurrently in the array.
    ztuT = singles.tile([128, B * N], BF)
    # v-natural per head: [n-in-tile, nt, head, v(64)|ones(1)|zeros(63)].
    # The AV stationary operand is the full [128, 128] block: 128 weight
    # columns keep fast-weight-load enabled so AV LDWEIGHTS stays hidden.
    ztuN = singles.tile([128, NTILES, HPER, NKT], BF)
    nc.gpsimd.memset(ztuN[:, :, :, DH:], 0.0)
    nc.vector.memset(ztuN[:, :, :, DH:DH + 1], 1.0)

    # ---- phase 1: proj1 (ztuT = W_g @ ZT^T) + phase 1.5: transposes (ztuN) ----
    def load_zin(b, split):
        zin = [zin_pool.tile([128, N], BF, tag="zin", name=f"zin{ci}")
               for ci in range(8)]
        if split:
            # jn-major order: chunk jn's 8 slices arrive before chunk jn+1's,
            # so proj1(0) can start after the first 8 DMAs instead of 29.
            # Split across the two HWDGE queues (Sync + Scalar, idle at
            # startup) so the per-DMA queue occupancy doesn't serialize.
            for jn in range(N // NQB):
                for ci in range(8):
                    eng = nc.scalar if ci % 2 else nc.sync
                    eng.dma_start(
                        out=zin[ci][:, jn * NQB:(jn + 1) * NQB],
                        in_=ztt[b, ci * 128:(ci + 1) * 128,
                                jn * NQB:(jn + 1) * NQB])
        else:
            for ci in range(8):
                nc.sync.dma_start(out=zin[ci],
                                  in_=ztt[b, ci * 128:(ci + 1) * 128, :])
        return zin

    def proj1_chunk(b, zin, jn):
        p1 = sc_pool.tile([128, 2 * NQB], F32, tag="sc", name="p1")
        p1v = p1[:, 0:NQB]
        for ci in range(8):
            nc.tensor.matmul(
                p1v, lhsT=wgt_sb[:, ci, :],
                rhs=zin[ci][:, jn * NQB:(jn + 1) * NQB],
                start=(ci == 0), stop=(ci == 7),
            )
        nc.vector.tensor_copy(
            out=ztuT[:, b * N + jn * NQB: b * N + (jn + 1) * NQB], in_=p1v)

    def transpose_chunk(b, jn):
        # PE transposes of 4 n-tiles into one pt psum tile, then a single
        # strided DVE copy into ztuN's [nt, head, v] layout.
        nt0 = b * NTB + 4 * jn
        pt = sc_pool.tile([128, 4, HPER, DH], BF, tag="sc", name="pt")
        for i in range(4):
            nt = nt0 + i
            nc.tensor.transpose(
                pt[:, i], ztuT[:, nt * NKT:(nt + 1) * NKT], ident)
        nc.vector.tensor_copy(out=ztuN[:, nt0:nt0 + 4, :, 0:DH], in_=pt)

    # ---- phase 2 defs: attention + proj2, software-pipelined across q-blocks ----
    exp_counter = [0]

    def attention_block(b, jq, filler=None, drain=None,
                        exp_every=None):
        if exp_every is None:
            exp_every = DVE_EXP_EVERY
        # Scores ring has 3 slots so the PE's QK runs ahead of exp; group
        # g+1's QK is emitted before group g's AV so the in-order PE queue
        # never waits on exp latency. drain(t) emits one proj2 chunk of the
        # PREVIOUS q-block at groups 1..4, spreading its PSUM->SBUF casts
        # across the block instead of bursting them into the Vector FIFO.
        q0 = b * N + jq * NQB
        avs = [av_pool.tile([128, NQB], F32, tag="av", name=f"av{h}")
               for h in range(HPER)]

        def emit_avs(g, exs):
            for hh in range(HPER):
                for u in range(2):
                    ik = 2 * g + u
                    vT = ztuN[:, b * NTB + ik, hh, :]
                    nc.tensor.matmul(avs[hh], lhsT=vT,
                                     rhs=exs[hh][:, u * NQB:(u + 1) * NQB],
                                     start=(ik == 0), stop=(ik == NTB - 1))

        prev = None
        for g in range(NTB // 2):               # groups of 2 key tiles
            scs = [sc_pool.tile([128, 2 * NQB], F32, tag="sc", name=f"sc{hh}")
                   for hh in range(HPER)]
            # interleave heads per key tile so the two K=64 row-packed
            # matmuls are adjacent in the PE queue (concurrent execution)
            for u in range(2):
                ik = 2 * g + u
                for hh in range(HPER):
                    p0 = hh * DH
                    kT = ztuT[p0:p0 + DH, b * N + ik * NKT: b * N + (ik + 1) * NKT]
                    qT = ztuT[p0:p0 + DH, q0:q0 + NQB]
                    nc.tensor.matmul(scs[hh][:, u * NQB:(u + 1) * NQB],
                                     lhsT=kT, rhs=qT, start=True, stop=True)
            exs = []
            for hh in range(HPER):
                use_dve = (exp_every > 0 and
                           exp_counter[0] % exp_every == exp_every - 1)
                exp_counter[0] += 1
                if use_dve:
                    exi = ex_pool.tile([128, 2 * NQB], I16, tag="ex", name="exi")
                    nc.vector.tensor_scalar(
                        out=exi, in0=scs[hh],
                        scalar1=A_SCH, scalar2=B_SCH,
                        op0=mybir.AluOpType.mult, op1=mybir.AluOpType.add)
                    exs.append(exi.bitcast(BF))
                else:
                    ex = ex_pool.tile([128, 2 * NQB], BF, tag="ex", name="ex")
                    nc.scalar.activation(
                        out=ex, in_=scs[hh],
                        func=mybir.ActivationFunctionType.Exp, scale=SCALE)
                    exs.append(ex)
            if prev is not None:
                emit_avs(*prev)
            if drain is not None and 1 <= g <= 3:
                drain(g - 1)
            prev = (g, exs)
        if filler is not None:
            filler()
        emit_avs(*prev)
        # last proj2 chunk of the previous block lands here: its matmuls
        # give the PE work to chew while finish_norm's dn copies (ACT) run.
        if drain is not None:
            drain(3)
        return avs

    def finish_norm(b, jq, avs):
        # softmax denominators -> per-head broadcast -> reciprocal -> scale.
        # Emitted at block END: dn extraction runs on the (idle) Scalar
        # engine so the bc matmul never waits on the Vector FIFO.
        nc.scalar.copy(out=dn[0:1, :], in_=avs[0][DH:DH + 1, :])
        nc.scalar.copy(out=dn[32:33, :], in_=avs[1][DH:DH + 1, :])
        bc = sc_pool.tile([128, 2 * NQB], F32, tag="sc", name="bc")
        bcv = bc[:, 0:NQB]
        nc.tensor.matmul(bcv, lhsT=sel, rhs=dn, start=True, stop=True)
        rc = rc_pool.tile([128, NQB], F32)
        nc.vector.reciprocal_approx_fast(out=rc, in_=bcv)
        sn = sn_pool.tile([128, NQB], BF)
        nc.vector.tensor_tensor(
            out=sn[0:64, :], in0=avs[0][0:DH, :], in1=rc[0:64, :],
            op=mybir.AluOpType.mult)
        nc.vector.tensor_tensor(
            out=sn[64:128, :], in0=avs[1][0:DH, :], in1=rc[64:128, :],
            op=mybir.AluOpType.mult)
        return sn

    def proj2_chunk(b, jq, sn, t, cast_on_act=False):
        # proj2: out[q, :] += ssa_norm_g @ W_g  (both heads contracted).
        # Both 512-column halves land in one 2-bank psum tile -> one bf16
        # cast and one contiguous (2KB/row) DMA store per 128 query rows.
        p2 = sc_pool.tile([128, 2 * NQB], F32, tag="sc", name="p2")
        for ch in range(2):
            nc.tensor.matmul(
                p2[:, ch * 512:(ch + 1) * 512],
                lhsT=sn[:, t * 128:(t + 1) * 128],
                rhs=wg_sb[:, ch * 512:(ch + 1) * 512],
                start=True, stop=True)
        p2s = rc_pool.tile([128, 1024], BF, tag="p2s")
        if cast_on_act:
            nc.scalar.copy(out=p2s, in_=p2)
        else:
            nc.vector.tensor_copy(out=p2s, in_=p2)
        r0 = b * N + jq * NQB + t * 128
        nc.sync.dma_start(out=out[r0:r0 + 128, :], in_=p2s)

    # ---- main schedule: batch b's proj1/transposes are interleaved into
    # batch b-1's attention at q-block granularity. Each q-block's norm is
    # emitted at its own block end; its proj2 chunks drain one-at-a-time
    # into the next q-block's group loop. ----
    state = {"sn": None}

    def make_drain():
        if state["sn"] is None:
            return None
        pb, pj, psn = state["sn"]
        return lambda t: proj2_chunk(pb, pj, psn, t)

    def flush_proj2():
        # final-block drain: alternate the casts between ACT and DVE so the
        # tail's four PSUM->SBUF casts run on two engines instead of one
        if state["sn"] is not None:
            pb, pj, psn = state["sn"]
            for t in range(NQB // 128):
                proj2_chunk(pb, pj, psn, t, cast_on_act=(t % 2 == 1))
            state["sn"] = None

    def attention_batch(b, filler=None, exp_every=None):
        for jq in range(N // NQB):
            fl = (lambda jq=jq: filler(jq)) if filler is not None else None
            avs = attention_block(b, jq, fl, drain=make_drain(),
                                  exp_every=exp_every)
            state["sn"] = (b, jq, finish_norm(b, jq, avs))

    # PE warm-up spin: ~8us of matmuls so the HAM clock gate is already at
    # 8/8 when the first DMA-gated proj1 matmul lands. Depends only on a
    # tiny DVE memset (NOT ident, whose affine_select sits behind the big
    # gpsimd ztuN memset) so it starts ~7us earlier.
    warmw = singles.tile([128, 32], BF)
    nc.vector.memset(warmw, 0.0)
    warm = sc_pool.tile([128, 2 * NQB], F32, tag="sc", name="warm")
    for _ in range(192):
        nc.tensor.matmul(warm[0:32, 0:32], lhsT=warmw, rhs=warmw,
                         start=True, stop=True)
    del warm

    zs = {0: load_zin(0, split=True)}
    for b in range(B):
        if b + 1 < B:
            zs[b + 1] = load_zin(b + 1, split=False)
        if b == 0:
            for jn in range(N // NQB):
                proj1_chunk(0, zs[0], jn)
                transpose_chunk(0, jn)
                # spin bursts between DMA-gated chunks keep each PE idle
                # window under the ~3.4us HAM re-throttle threshold
                wsp = sc_pool.tile([128, 2 * NQB], F32, tag="sc", name="wsp")
                for _ in range(48):
                    nc.tensor.matmul(wsp[0:32, 0:32], lhsT=warmw, rhs=warmw,
                                     start=True, stop=True)
                del wsp
            zs.pop(0)
        else:
            zin = zs.pop(b)

            def filler(jq, b=b, zin=zin):
                proj1_chunk(b, zin, jq)
                transpose_chunk(b, jq)

            attention_batch(b - 1, filler)
    attention_batch(B - 1)
    flush_proj2()


def _get_nc():
    if "nc" not in _CACHE:
        _CACHE["nc"] = _build_kernel()
    return _CACHE["nc"]


def kernel(ZT: np.ndarray, W: np.ndarray) -> np.ndarray:
    ZT = np.asarray(ZT, dtype=np.float32)
    W = np.asarray(W, dtype=np.float32)
    ztt = np.ascontiguousarray(ZT.transpose(0, 2, 1)).astype(ml_dtypes.bfloat16)
    in_maps = []
    for c in range(8):
        wgf = W[c * KP:(c + 1) * KP, :]
        in_maps.append({
            "ztt": ztt,
            "wgt": np.ascontiguousarray(wgf.T).astype(ml_dtypes.bfloat16),
            "wg": np.ascontiguousarray(wgf).astype(ml_dtypes.bfloat16),
        })
    nc = _get_nc()
    res = run_bass_kernel_spmd(nc, in_maps, core_ids=list(range(8)))
    acc = np.zeros((B * N, C), dtype=np.float32)
    for r in res.results:
        acc += np.asarray(r["out"], dtype=np.float32)
    return acc.reshape(B, N, C)


if __name__ == "__main__":
    rng = np.random.default_rng(0)
    zt = rng.standard_normal((B, N, C), dtype=np.float32)
    w = rng.standard_normal((KP * 8, C), dtype=np.float32) * C ** -0.5
    o = kernel(zt, w)
    print("out", o.shape, o.dtype, float(np.abs(o).mean()))


# revision 31
# speedup vs baseline: 1.1830x; 1.1830x over previous
"""Trainium2 Bass kernel for nn_Attention_Encode (B=4, N=2048, DIM=1024, H=16, DH=64).

Sharding: 16 heads -> 8 cores x 2 heads (tensor parallel). Each core computes
  ztu_g = W_g @ ZT^T          (its 128 output channels = 2 heads)
  attention per (batch, head) with Q=K=V=ztu
  partial_out = ssa_g @ W_g   (row-sharded output projection)
Host sums the 8 partials (the all-reduce step of a row-sharded projection).

On-device layout is fully transposed ("scoresT" = [keys, queries]) so that
softmax needs no transposes: the AV matmul's stationary operand [V | ones]
produces both the numerator and the softmax denominator.

v3 changes vs baseline:
 - QK matmuls are K=64 row-packed via tile_position: head A occupies PE
   rows 0:63, head B rows 64:127, the two matmuls run concurrently in the
   array (2x QK throughput; ztuT holds both heads stacked, unpadded).
 - Scores ring has 3 slots (6 PSUM banks); proj1/proj2/bc/warm share the
   same ring so the whole kernel fits 8 banks while QK can run 2 groups
   ahead of exp -> ACT stays saturated, PE never gates on exp latency.
 - A tunable fraction of exp tiles runs on the Vector engine as a
   Schraudolph fast-exp (one fused mult+add tensor_scalar emitting the
   bf16 bit pattern via an int16 store), offloading the saturated ACT.
 - ztuN (v-natural) is built by DMA xbar transposes (no PE/DVE involved);
   AV stationary operand is [v(64) | ones] with M=65.
 - proj2 writes both 512-column halves into one 2-bank PSUM tile: one
   bf16 cast, one contiguous 2KB-per-row DMA store per 128 rows.
"""
import os
import sys

for _p in ('/opt/trn_rl_repo',):
    if _p not in sys.path:
        sys.path.insert(0, _p)

from contextlib import ExitStack

import numpy as np
import ml_dtypes

import concourse.bacc as bacc
import concourse.mybir as mybir
import concourse.tile as tile
from concourse.bass_utils import run_bass_kernel_spmd
from concourse.masks import make_identity

B, N, C = 4, 2048, 1024          # batch, seq, model dim
KP, DH, HPER = 128, 64, 2        # per-core channels, head dim, heads per core
NQB = 512                        # query block
NKT = 128                        # key tile
NTB = N // NKT                   # 16 key tiles per batch
NTILES = B * NTB                 # 64 n-tiles total
SCALE = DH ** -0.5               # 0.125
BF = mybir.dt.bfloat16
F32 = mybir.dt.float32
F32R = mybir.dt.float32r
I16 = mybir.dt.int16

# Schraudolph fast-exp (bf16 bit pattern built by one fp32 mult+add):
#   bits_i16 = round(score * A_SCH + B_SCH);  bitcast -> bf16 ~ exp(score*SCALE)
LOG2E = 1.4426950408889634
SIGMA_SCH = 0.0579               # minimax offset, |rel err| <= ~4%
A_SCH = SCALE * LOG2E * 128.0
B_SCH = (127.0 - SIGMA_SCH) * 128.0
DVE_EXP_EVERY = int(os.environ.get("DVE_EXP_EVERY", "3"))

_CACHE = {}


def _build_kernel():
    nc = bacc.Bacc("TRN2", target_bir_lowering=False, debug=False)
    ztt = nc.dram_tensor("ztt", [B, C, N], BF, kind="ExternalInput").ap()
    wgt = nc.dram_tensor("wgt", [C, KP], BF, kind="ExternalInput").ap()   # W_g^T
    wg = nc.dram_tensor("wg", [KP, C], BF, kind="ExternalInput").ap()     # W_g
    out = nc.dram_tensor("out", [B * N, C], BF, kind="ExternalOutput").ap()

    with tile.TileContext(nc) as tc, ExitStack() as ctx:
        _body(ctx, tc, ztt, wgt, wg, out)
    nc.compile()
    return nc


def _body(ctx, tc, ztt, wgt, wg, out):
    nc = tc.nc
    singles = ctx.enter_context(tc.tile_pool(name="singles", bufs=1))
    zin_pool = ctx.enter_context(tc.tile_pool(name="zin", bufs=16))
    sc_pool = ctx.enter_context(tc.tile_pool(name="sc", bufs=3, space="PSUM"))
    av_pool = ctx.enter_context(tc.tile_pool(name="av", bufs=2, space="PSUM"))
    ex_pool = ctx.enter_context(tc.tile_pool(name="ex", bufs=12))
    sn_pool = ctx.enter_context(tc.tile_pool(name="sn", bufs=4))
    rc_pool = ctx.enter_context(tc.tile_pool(name="rc", bufs=4))

    # ---- persistent SBUF ----
    wgt_sb = singles.tile([128, 8, KP], BF)            # [c-in-tile, ci, k]
    nc.sync.dma_start(out=wgt_sb, in_=wgt.rearrange("(ci p) k -> p ci k", p=128))
    wg_sb = singles.tile([KP, C], BF)
    nc.sync.dma_start(out=wg_sb, in_=wg)
    ident = singles.tile([128, 128], BF)
    make_identity(nc, ident)
    sel = singles.tile([128, 128], BF)                 # den -> per-head row broadcast
    nc.vector.memset(sel, 0.0)
    nc.vector.memset(sel[0:1, 0:64], 1.0)
    nc.vector.memset(sel[32:33, 64:128], 1.0)
    dn = singles.tile([128, NQB], BF)                  # dens: head A row 0, head B row 32
    nc.vector.memset(dn, 0.0)
    # ztu^T with both heads stacked: rows 0:64 = head A, rows 64:128 = head B.
    # QK matmuls use K=64 slices; tile_position row-packing (auto-derived from
    # base partitions) runs the two heads' matmuls concurrently in the array.
    ztuT = singles.tile([128, B * N], BF)
    # v-natural per head: [n-in-tile, nt, head, v(64)|ones(1)|zeros(63)].
    # The AV stationary operand is the full [128, 128] block: 128 weight
    # columns keep fast-weight-load enabled so AV LDWEIGHTS stays hidden.
    ztuN = singles.tile([128, NTILES, HPER, NKT], BF)
    nc.gpsimd.memset(ztuN[:, :, :, DH:], 0.0)
    nc.vector.memset(ztuN[:, :, :, DH:DH + 1], 1.0)

    # ---- phase 1: proj1 (ztuT = W_g @ ZT^T) + phase 1.5: transposes (ztuN) ----
    def load_zin(b, split):
        zin = [zin_pool.tile([128, N], BF, tag="zin", name=f"zin{ci}")
               for ci in range(8)]
        if split:
            # jn-major order: chunk jn's 8 slices arrive before chunk jn+1's,
            # so proj1(0) can start after the first 8 DMAs instead of 29.
            for jn in range(N // NQB):
                for ci in range(8):
                    nc.sync.dma_start(
                        out=zin[ci][:, jn * NQB:(jn + 1) * NQB],
                        in_=ztt[b, ci * 128:(ci + 1) * 128,
                                jn * NQB:(jn + 1) * NQB])
        else:
            for ci in range(8):
                nc.sync.dma_start(out=zin[ci],
                                  in_=ztt[b, ci * 128:(ci + 1) * 128, :])
        return zin

    def proj1_chunk(b, zin, jn):
        p1 = sc_pool.tile([128, 2 * NQB], F32, tag="sc", name="p1")
        p1v = p1[:, 0:NQB]
        for ci in range(8):
            nc.tensor.matmul(
                p1v, lhsT=wgt_sb[:, ci, :],
                rhs=zin[ci][:, jn * NQB:(jn + 1) * NQB],
                start=(ci == 0), stop=(ci == 7),
            )
        nc.vector.tensor_copy(
            out=ztuT[:, b * N + jn * NQB: b * N + (jn + 1) * NQB], in_=p1v)

    def transpose_chunk(b, jn):
        # PE transposes of 4 n-tiles into one pt psum tile, then a single
        # strided DVE copy into ztuN's [nt, head, v] layout.
        nt0 = b * NTB + 4 * jn
        pt = sc_pool.tile([128, 4, HPER, DH], BF, tag="sc", name="pt")
        for i in range(4):
            nt = nt0 + i
            nc.tensor.transpose(
                pt[:, i], ztuT[:, nt * NKT:(nt + 1) * NKT], ident)
        nc.vector.tensor_copy(out=ztuN[:, nt0:nt0 + 4, :, 0:DH], in_=pt)

    # ---- phase 2 defs: attention + proj2, software-pipelined across q-blocks ----
    exp_counter = [0]

    def attention_block(b, jq, filler=None, drain=None,
                        exp_every=None):
        if exp_every is None:
            exp_every = DVE_EXP_EVERY
        # Scores ring has 3 slots so the PE's QK runs ahead of exp; group
        # g+1's QK is emitted before group g's AV so the in-order PE queue
        # never waits on exp latency. drain(t) emits one proj2 chunk of the
        # PREVIOUS q-block at groups 1..4, spreading its PSUM->SBUF casts
        # across the block instead of bursting them into the Vector FIFO.
        q0 = b * N + jq * NQB
        avs = [av_pool.tile([128, NQB], F32, tag="av", name=f"av{h}")
               for h in range(HPER)]

        def emit_avs(g, exs):
            for hh in range(HPER):
                for u in range(2):
                    ik = 2 * g + u
                    vT = ztuN[:, b * NTB + ik, hh, :]
                    nc.tensor.matmul(avs[hh], lhsT=vT,
                                     rhs=exs[hh][:, u * NQB:(u + 1) * NQB],
                                     start=(ik == 0), stop=(ik == NTB - 1))

        prev = None
        for g in range(NTB // 2):               # groups of 2 key tiles
            scs = [sc_pool.tile([128, 2 * NQB], F32, tag="sc", name=f"sc{hh}")
                   for hh in range(HPER)]
            # interleave heads per key tile so the two K=64 row-packed
            # matmuls are adjacent in the PE queue (concurrent execution)
            for u in range(2):
                ik = 2 * g + u
                for hh in range(HPER):
                    p0 = hh * DH
                    kT = ztuT[p0:p0 + DH, b * N + ik * NKT: b * N + (ik + 1) * NKT]
                    qT = ztuT[p0:p0 + DH, q0:q0 + NQB]
                    nc.tensor.matmul(scs[hh][:, u * NQB:(u + 1) * NQB],
                                     lhsT=kT, rhs=qT, start=True, stop=True)
            exs = []
            for hh in range(HPER):
                use_dve = (exp_every > 0 and
                           exp_counter[0] % exp_every == exp_every - 1)
                exp_counter[0] += 1
                if use_dve:
                    exi = ex_pool.tile([128, 2 * NQB], I16, tag="ex", name="exi")
                    nc.vector.tensor_scalar(
                        out=exi, in0=scs[hh],
                        scalar1=A_SCH, scalar2=B_SCH,
                        op0=mybir.AluOpType.mult, op1=mybir.AluOpType.add)
                    exs.append(exi.bitcast(BF))
                else:
                    ex = ex_pool.tile([128, 2 * NQB], BF, tag="ex", name="ex")
                    nc.scalar.activation(
                        out=ex, in_=scs[hh],
                        func=mybir.ActivationFunctionType.Exp, scale=SCALE)
                    exs.append(ex)
            if prev is not None:
                emit_avs(*prev)
            if drain is not None and 1 <= g <= 3:
                drain(g - 1)
            prev = (g, exs)
        if filler is not None:
            filler()
        emit_avs(*prev)
        # last proj2 chunk of the previous block lands here: its matmuls
        # give the PE work to chew while finish_norm's dn copies (ACT) run.
        if drain is not None:
            drain(3)
        return avs

    def finish_norm(b, jq, avs):
        # softmax denominators -> per-head broadcast -> reciprocal -> scale.
        # Emitted at block END: dn extraction runs on the (idle) Scalar
        # engine so the bc matmul never waits on the Vector FIFO.
        nc.scalar.copy(out=dn[0:1, :], in_=avs[0][DH:DH + 1, :])
        nc.scalar.copy(out=dn[32:33, :], in_=avs[1][DH:DH + 1, :])
        bc = sc_pool.tile([128, 2 * NQB], F32, tag="sc", name="bc")
        bcv = bc[:, 0:NQB]
        nc.tensor.matmul(bcv, lhsT=sel, rhs=dn, start=True, stop=True)
        rc = rc_pool.tile([128, NQB], F32)
        nc.vector.reciprocal_approx_fast(out=rc, in_=bcv)
        sn = sn_pool.tile([128, NQB], BF)
        nc.vector.tensor_tensor(
            out=sn[0:64, :], in0=avs[0][0:DH, :], in1=rc[0:64, :],
            op=mybir.AluOpType.mult)
        nc.vector.tensor_tensor(
            out=sn[64:128, :], in0=avs[1][0:DH, :], in1=rc[64:128, :],
            op=mybir.AluOpType.mult)
        return sn

    def proj2_chunk(b, jq, sn, t):
        # proj2: out[q, :] += ssa_norm_g @ W_g  (both heads contracted).
        # Both 512-column halves land in one 2-bank psum tile -> one bf16
        # cast and one contiguous (2KB/row) DMA store per 128 query rows.
        p2 = sc_pool.tile([128, 2 * NQB], F32, tag="sc", name="p2")
        for ch in range(2):
            nc.tensor.matmul(
                p2[:, ch * 512:(ch + 1) * 512],
                lhsT=sn[:, t * 128:(t + 1) * 128],
                rhs=wg_sb[:, ch * 512:(ch + 1) * 512],
                start=True, stop=True)
        p2s = rc_pool.tile([128, 1024], BF, tag="p2s")
        nc.vector.tensor_copy(out=p2s, in_=p2)
        r0 = b * N + jq * NQB + t * 128
        nc.sync.dma_start(out=out[r0:r0 + 128, :], in_=p2s)

    # ---- main schedule: batch b's proj1/transposes are interleaved into
    # batch b-1's attention at q-block granularity. Each q-block's norm is
    # emitted at its own block end; its proj2 chunks drain one-at-a-time
    # into the next q-block's group loop. ----
    state = {"sn": None}

    def make_drain():
        if state["sn"] is None:
            return None
        pb, pj, psn = state["sn"]
        return lambda t: proj2_chunk(pb, pj, psn, t)

    def flush_proj2():
        if state["sn"] is not None:
            pb, pj, psn = state["sn"]
            for t in range(NQB // 128):
                proj2_chunk(pb, pj, psn, t)
            state["sn"] = None

    def attention_batch(b, filler=None, exp_every=None):
        for jq in range(N // NQB):
            fl = (lambda jq=jq: filler(jq)) if filler is not None else None
            avs = attention_block(b, jq, fl, drain=make_drain(),
                                  exp_every=exp_every)
            state["sn"] = (b, jq, finish_norm(b, jq, avs))

    # PE warm-up spin: ~8us of matmuls so the HAM clock gate is already at
    # 8/8 when the first DMA-gated proj1 matmul lands. Depends only on a
    # tiny DVE memset (NOT ident, whose affine_select sits behind the big
    # gpsimd ztuN memset) so it starts ~7us earlier.
    warmw = singles.tile([128, 32], BF)
    nc.vector.memset(warmw, 0.0)
    warm = sc_pool.tile([128, 2 * NQB], F32, tag="sc", name="warm")
    for _ in range(192):
        nc.tensor.matmul(warm[0:32, 0:32], lhsT=warmw, rhs=warmw,
                         start=True, stop=True)
    del warm

    zs = {0: load_zin(0, split=True)}
    for b in range(B):
        if b + 1 < B:
            zs[b + 1] = load_zin(b + 1, split=False)
        if b == 0:
            for jn in range(N // NQB):
                proj1_chunk(0, zs[0], jn)
                transpose_chunk(0, jn)
                # spin bursts between DMA-gated chunks keep each PE idle
                # window under the ~3.4us HAM re-throttle threshold
                wsp = sc_pool.tile([128, 2 * NQB], F32, tag="sc", name="wsp")
                for _ in range(48):
                    nc.tensor.matmul(wsp[0:32, 0:32], lhsT=warmw, rhs=warmw,
                                     start=True, stop=True)
                del wsp
            zs.pop(0)
        else:
            zin = zs.pop(b)

            def filler(jq, b=b, zin=zin):
                proj1_chunk(b, zin, jq)
                transpose_chunk(b, jq)

            attention_batch(b - 1, filler)
    attention_batch(B - 1)
    flush_proj2()


def _get_nc():
    if "nc" not in _CACHE:
        _CACHE["nc"] = _build_kernel()
    return _CACHE["nc"]


def kernel(ZT: np.ndarray, W: np.ndarray) -> np.ndarray:
    ZT = np.asarray(ZT, dtype=np.float32)
    W = np.asarray(W, dtype=np.float32)
    ztt = np.ascontiguousarray(ZT.transpose(0, 2, 1)).astype(ml_dtypes.bfloat16)
    in_maps = []
    for c in range(8):
        wgf = W[c * KP:(c + 1) * KP, :]
        in_maps.append({
            "ztt": ztt,
            "wgt": np.ascontiguousarray(wgf.T).astype(ml_dtypes.bfloat16),
            "wg": np.ascontiguousarray(wgf).astype(ml_dtypes.bfloat16),
        })
    nc = _get_nc()
    res = run_bass_kernel_spmd(nc, in_maps, core_ids=list(range(8)))
    acc = np.zeros((B * N, C), dtype=np.float32)
    for r in res.results:
        acc += np.asarray(r["out"], dtype=np.float32)
    return acc.reshape(B, N, C)


if __name__ == "__main__":
    rng = np.random.default_rng(0)
    zt = rng.standard_normal((B, N, C), dtype=np.float32)
    w = rng.standard_normal((KP * 8, C), dtype=np.float32) * C ** -0.5
    o = kernel(zt, w)
    print("out", o.shape, o.dtype, float(np.abs(o).mean()))
